# revision 5
# baseline (speedup 1.0000x reference)
"""Multi-head attention + residual + LayerNorm on 8 Trainium2 NeuronCores.

Sharding: data-parallel over (batch, query-half). Core c handles batch c//2,
query rows [(c%2)*1024, (c%2)*1024+1024). K/V projections for a batch are
duplicated across the 2 cores sharing it (no collectives needed).

Device dataflow (per core), everything feature-major where needed so no
on-device transposes are required (inputs are pre-transposed on host):
  1. q_feat[e, q]  = sum_d W_Q^T[d, e] * Q^T[d, q]      (f32r matmuls, fp16 out)
  2. v[ks, e]      = sum_d V^T[d, ks]  * W_V^T[d, e]    (fp16, kept in SBUF,
     with a ones-column appended per head-pair for softmax denominators)
  3. k_feat[e, ks] = sum_d W_K^T[d, e] * K^T[d, ks]     (fp16, spilled to DRAM)
  4. per head-pair (A = rows 0-63 and B = 64-127 of the pair's e-slab):
     S^T[ks, q] = k_feat^T-slab . q_feat-slab  (row-packed K=64 matmul pairs)
     P^T = exp(S^T/8) * keep    (ACT exp + DVE mask multiply, fp16)
     ctx^T/denom via PV matmuls with lhsT = [v_head | ones] (m=65);
     divide by denom (broadcast via K=1 ones matmul).
  5. out[s, :] = LayerNorm(ctx_norm^T . W_fc^T + Q_residual)
"""

import numpy as np
from contextlib import ExitStack

import concourse.bass as bass
import concourse.mybir as mybir
import concourse.tile as tile
from concourse import bacc
from concourse.bass_utils import run_bass_kernel_spmd

F32 = mybir.dt.float32
F32R = mybir.dt.float32r
F16 = mybir.dt.float16
U8 = mybir.dt.uint8
EXP = mybir.ActivationFunctionType.Exp
SQRT = mybir.ActivationFunctionType.Sqrt

B, S, D, H, DK = 4, 2048, 1024, 16, 64
P = 128
SQ = 1024                # query rows per core
NDT = D // P             # 8 contraction tiles
NPR = H // 2             # 8 head pairs (= e-tiles)
NKT = S // P             # 16 key tiles
NQC = SQ // 512          # 2 query chunks of 512
EPS = 1e-5

_NC_CACHE = None


def _build_nc():
    nc = bacc.Bacc("TRN2", target_bir_lowering=False, debug=False, num_devices=8)

    qt = nc.dram_tensor("qt", [D, SQ], F32, kind="ExternalInput")      # Q^T slice
    kin = nc.dram_tensor("kin", [D, S], F32, kind="ExternalInput")     # K^T
    vin = nc.dram_tensor("vin", [D, S], F32, kind="ExternalInput")     # V^T
    qres = nc.dram_tensor("qres", [SQ, D], F32, kind="ExternalInput")  # Q row-major slice
    keept = nc.dram_tensor("keept", [S, SQ], U8, kind="ExternalInput")  # ~mask, [ks, q]
    wqt = nc.dram_tensor("wqt", [D, D], F32, kind="ExternalInput")     # W_Q.T
    wkt = nc.dram_tensor("wkt", [D, D], F32, kind="ExternalInput")
    wvt = nc.dram_tensor("wvt", [D, D], F32, kind="ExternalInput")
    wft = nc.dram_tensor("wft", [D, D], F32, kind="ExternalInput")
    out = nc.dram_tensor("out", [SQ, D], F32, kind="ExternalOutput")

    qt_r = qt.ap().rearrange("(dt p) q -> p dt q", p=P)
    kin_r = kin.ap().rearrange("(dt p) s -> p dt s", p=P)
    vin_r = vin.ap().rearrange("(dt p) s -> p dt s", p=P)
    keept_r = keept.ap().rearrange("(t p) q -> p t q", p=P)

    with tile.TileContext(nc) as tc, ExitStack() as top:
        dram = top.enter_context(tc.tile_pool(name="dram", bufs=1, space="DRAM"))
        k_spill = dram.tile([D, S], F16)
        ctx_spill = dram.tile([D, SQ], F32R)

        with ExitStack() as attn_scope:
            persist = attn_scope.enter_context(tc.tile_pool(name="persist", bufs=1))
            q_feat = persist.tile([P, NPR, SQ], F16)
            v_sl = persist.tile([P, NKT, NPR, 130], F16)
            keep16 = persist.tile([P, NKT, SQ], F16)
            ones1 = persist.tile([1, 64], F32R)
            ones1_f32 = persist.tile([1, 64], F32)
            nc.vector.memset(ones1_f32[:], 1.0)
            nc.vector.tensor_copy(ones1[:], ones1_f32[:])
            nc.vector.memset(v_sl[:, :, :, 64:65], 1.0)
            nc.vector.memset(v_sl[:, :, :, 129:130], 1.0)

            # keep mask: load u8, convert to fp16
            with tc.tile_pool(name="kload", bufs=1) as kload:
                kp_u8 = kload.tile([P, NKT, SQ], U8)
                nc.sync.dma_start(kp_u8[:], keept_r)
                for t in range(NKT):
                    nc.vector.tensor_copy(keep16[:, t, :], kp_u8[:, t, :])

            # ---- stage 1: q projection -> q_feat (fp16, resident) ----
            with tc.tile_pool(name="w1", bufs=1) as wpool, \
                 tc.tile_pool(name="x1", bufs=2) as xpool, \
                 tc.tile_pool(name="ps1", bufs=4, space="PSUM") as pspool:
                wq = wpool.tile([P, NDT, D], F32R)
                nc.gpsimd.dma_start(wq[:], wqt.ap().rearrange("(dt p) e -> p dt e", p=P))
                for qc in range(NQC):
                    qin_c = xpool.tile([P, NDT, 512], F32R, tag="qin")
                    nc.gpsimd.dma_start(qin_c[:], qt_r[:, :, qc * 512:(qc + 1) * 512])
                    for et in range(NPR):
                        ps = pspool.tile([P, 512], F32, tag="ps")
                        for dt_i in range(NDT):
                            nc.tensor.matmul(
                                ps[:],
                                lhsT=wq[:, dt_i, et * P:(et + 1) * P],
                                rhs=qin_c[:, dt_i, :],
                                start=(dt_i == 0), stop=(dt_i == NDT - 1))
                        nc.vector.tensor_copy(
                            q_feat[:, et, qc * 512:(qc + 1) * 512], ps[:])

            # ---- stage 2: v projection -> v_sl (fp16, resident, ones cols kept) ----
            with tc.tile_pool(name="w2", bufs=1) as wpool, \
                 tc.tile_pool(name="x2", bufs=2) as xpool, \
                 tc.tile_pool(name="ps2", bufs=4, space="PSUM") as pspool:
                wv = wpool.tile([P, NDT, D], F32R)
                nc.gpsimd.dma_start(wv[:], wvt.ap().rearrange("(dt p) e -> p dt e", p=P))
                for kt in range(NKT):
                    vin_c = xpool.tile([P, NDT, P], F32R, tag="vinc")
                    nc.gpsimd.dma_start(vin_c[:], vin_r[:, :, kt * P:(kt + 1) * P])
                    for ec in range(2):
                        ps = pspool.tile([P, 512], F32, tag="ps")
                        for dt_i in range(NDT):
                            nc.tensor.matmul(
                                ps[:],
                                lhsT=vin_c[:, dt_i, :],
                                rhs=wv[:, dt_i, ec * 512:(ec + 1) * 512],
                                start=(dt_i == 0), stop=(dt_i == NDT - 1))
                        ps_g = ps.rearrange("p (g c) -> p g c", c=P)
                        nc.vector.tensor_copy(
                            v_sl[:, kt, 4 * ec:4 * ec + 4, 0:64], ps_g[:, :, 0:64])
                        nc.vector.tensor_copy(
                            v_sl[:, kt, 4 * ec:4 * ec + 4, 65:129], ps_g[:, :, 64:128])

            # ---- stage 3: k projection -> k_spill (fp16, DRAM) ----
            with tc.tile_pool(name="w3", bufs=1) as wpool, \
                 tc.tile_pool(name="x3", bufs=2) as xpool, \
                 tc.tile_pool(name="st3", bufs=3) as stpool, \
                 tc.tile_pool(name="ps3", bufs=4, space="PSUM") as pspool:
                wk = wpool.tile([P, NDT, D], F32R)
                nc.gpsimd.dma_start(wk[:], wkt.ap().rearrange("(dt p) e -> p dt e", p=P))
                for kc in range(S // 512):
                    kin_c = xpool.tile([P, NDT, 512], F32R, tag="kinc")
                    nc.gpsimd.dma_start(kin_c[:], kin_r[:, :, kc * 512:(kc + 1) * 512])
                    for et in range(NPR):
                        ps = pspool.tile([P, 512], F32, tag="ps")
                        for dt_i in range(NDT):
                            nc.tensor.matmul(
                                ps[:],
                                lhsT=wk[:, dt_i, et * P:(et + 1) * P],
                                rhs=kin_c[:, dt_i, :],
                                start=(dt_i == 0), stop=(dt_i == NDT - 1))
                        stg = stpool.tile([P, 512], F16, tag="stg")
                        nc.vector.tensor_copy(stg[:], ps[:])
                        nc.sync.dma_start(
                            k_spill[et * P:(et + 1) * P, kc * 512:(kc + 1) * 512],
                            stg[:])

            # ---- stage 4: attention per head pair ----
            with tc.tile_pool(name="ksl", bufs=2) as kslp, \
                 tc.tile_pool(name="pT", bufs=2) as pTp, \
                 tc.tile_pool(name="ctxn", bufs=2) as ctxnp, \
                 tc.tile_pool(name="sm4", bufs=3) as smalls, \
                 tc.tile_pool(name="scps", bufs=2, space="PSUM") as scps, \
                 tc.tile_pool(name="ctxps", bufs=2, space="PSUM") as ctxps, \
                 tc.tile_pool(name="rdps", bufs=1, space="PSUM") as rdps:
                for pr in range(NPR):
                    k_sl = kslp.tile([P, S], F16, tag="ksl")
                    nc.sync.dma_start(k_sl[:], k_spill[pr * P:(pr + 1) * P, :])
                    pT_A = pTp.tile([P, NKT, SQ], F16, tag="pT")
                    pT_B = pTp.tile([P, NKT, SQ], F16, tag="pT")
                    for t in range(NKT):
                        sA = scps.tile([P, SQ], F32, tag="sc")
                        sB = scps.tile([P, SQ], F32, tag="sc")
                        for qc in range(NQC):
                            qs = slice(qc * 512, (qc + 1) * 512)
                            nc.tensor.matmul(
                                sA[:, qs],
                                lhsT=k_sl[0:64, t * P:(t + 1) * P],
                                rhs=q_feat[0:64, pr, qs],
                                start=True, stop=True, tile_position=(0, 0))
                            nc.tensor.matmul(
                                sB[:, qs],
                                lhsT=k_sl[64:128, t * P:(t + 1) * P],
                                rhs=q_feat[64:128, pr, qs],
                                start=True, stop=True, tile_position=(64, 0))
                        nc.scalar.activation(pT_A[:, t, :], sA[:], EXP, scale=0.125)
                        nc.scalar.activation(pT_B[:, t, :], sB[:], EXP, scale=0.125)
                        nc.vector.tensor_mul(pT_A[:, t, :], pT_A[:, t, :], keep16[:, t, :])
                        nc.vector.tensor_mul(pT_B[:, t, :], pT_B[:, t, :], keep16[:, t, :])

                    ctxn = ctxnp.tile([P, SQ], F32R, tag="ctxn")
                    for qc in range(NQC):
                        qs = slice(qc * 512, (qc + 1) * 512)
                        cA = ctxps.tile([65, 512], F32, tag="ctx")
                        cB = ctxps.tile([65, 512], F32, tag="ctx")
                        for t in range(NKT):
                            nc.tensor.matmul(
                                cA[:], lhsT=v_sl[:, t, pr, 0:65],
                                rhs=pT_A[:, t, qs],
                                start=(t == 0), stop=(t == NKT - 1))
                            nc.tensor.matmul(
                                cB[:], lhsT=v_sl[:, t, pr, 65:130],
                                rhs=pT_B[:, t, qs],
                                start=(t == 0), stop=(t == NKT - 1))
                        den_A = smalls.tile([1, 512], F32R, tag="denA")
                        den_B = smalls.tile([1, 512], F32R, tag="denB")
                        with nc.allow_low_precision(reason="f32r is 32-bit storage; rounding only"):
                            nc.vector.reciprocal(den_A[:], cA[64:65, :])
                            nc.vector.reciprocal(den_B[:], cB[64:65, :])
                        rd_a = rdps.tile([64, 512], F32, tag="rda")
                        rd_b = rdps.tile([64, 512], F32, tag="rdb")
                        nc.tensor.matmul(
                            rd_a[:], lhsT=ones1[:], rhs=den_A[:],
                            start=True, stop=True)
                        nc.tensor.matmul(
                            rd_b[:], lhsT=ones1[:], rhs=den_B[:],
                            start=True, stop=True)
                        rda_sb = smalls.tile([64, 512], F32, tag="rdasb")
                        rdb_sb = smalls.tile([64, 512], F32, tag="rdbsb")
                        nc.vector.tensor_copy(rda_sb[:], rd_a[:])
                        nc.vector.tensor_copy(rdb_sb[:], rd_b[:])
                        nc.vector.tensor_mul(ctxn[0:64, qs], cA[0:64, :], rda_sb[:])
                        nc.vector.tensor_mul(ctxn[64:128, qs], cB[0:64, :], rdb_sb[:])
                    nc.sync.dma_start(ctx_spill[pr * P:(pr + 1) * P, :], ctxn[:])

        # ---- stage 5: fc + residual + LayerNorm ----
        ctx_r = ctx_spill.rearrange("(dt p) q -> p dt q", p=P)
        with tc.tile_pool(name="w5", bufs=1) as wpool, \
             tc.tile_pool(name="x5", bufs=2) as xpool, \
             tc.tile_pool(name="o5", bufs=2) as opool, \
             tc.tile_pool(name="ln5", bufs=4) as lnpool, \
             tc.tile_pool(name="ps5", bufs=4, space="PSUM") as pspool:
            wf = wpool.tile([P, NDT, D], F32R)
            nc.gpsimd.dma_start(wf[:], wft.ap().rearrange("(dt p) e -> p dt e", p=P))
            eps_t = wpool.tile([P, 1], F32)
            nc.vector.memset(eps_t[:], EPS)
            for st in range(SQ // P):
                ctx_sb = xpool.tile([P, NDT, P], F32R, tag="ctxsb")
                nc.sync.dma_start(ctx_sb[:], ctx_r[:, :, st * P:(st + 1) * P])
                qr_sb = xpool.tile([P, D], F32, tag="qr")
                nc.sync.dma_start(qr_sb[:], qres.ap()[st * P:(st + 1) * P, :])
                out_sb = opool.tile([P, D], F32, tag="osb")
                for ec in range(2):
                    ps = pspool.tile([P, 512], F32, tag="ps")
                    for dt_i in range(NDT):
                        nc.tensor.matmul(
                            ps[:],
                            lhsT=ctx_sb[:, dt_i, :],
                            rhs=wf[:, dt_i, ec * 512:(ec + 1) * 512],
                            start=(dt_i == 0), stop=(dt_i == NDT - 1))
                    nc.vector.tensor_add(
                        out_sb[:, ec * 512:(ec + 1) * 512], ps[:],
                        qr_sb[:, ec * 512:(ec + 1) * 512])
                stats = lnpool.tile([P, 2, 6], F32, tag="stats")
                nc.vector.bn_stats(stats[:, 0, :], out_sb[:, 0:512])
                nc.vector.bn_stats(stats[:, 1, :], out_sb[:, 512:1024])
                mv = lnpool.tile([P, 2], F32, tag="mv")
                nc.vector.bn_aggr(mv[:], stats[:])
                rstd = lnpool.tile([P, 1], F32, tag="rstd")
                nc.scalar.activation(rstd[:], mv[:, 1:2], SQRT, bias=eps_t[:])
                nc.vector.reciprocal(rstd[:], rstd[:])
                nc.vector.tensor_scalar(
                    out_sb[:], out_sb[:],
                    scalar1=mv[:, 0:1], scalar2=rstd[:],
                    op0=mybir.AluOpType.subtract, op1=mybir.AluOpType.mult)
                nc.sync.dma_start(out.ap()[st * P:(st + 1) * P, :], out_sb[:])

    nc.compile()
    return nc


def _get_nc():
    global _NC_CACHE
    if _NC_CACHE is None:
        _NC_CACHE = _build_nc()
    return _NC_CACHE


def _make_in_maps(Q, K, V, attn_mask, W_Q, W_K, W_V, W_fc):
    Q = np.asarray(Q, dtype=np.float32)
    K = np.asarray(K, dtype=np.float32)
    V = np.asarray(V, dtype=np.float32)
    attn_mask = np.asarray(attn_mask)
    wqt = np.ascontiguousarray(np.asarray(W_Q, np.float32).T)
    wkt = np.ascontiguousarray(np.asarray(W_K, np.float32).T)
    wvt = np.ascontiguousarray(np.asarray(W_V, np.float32).T)
    wft = np.ascontiguousarray(np.asarray(W_fc, np.float32).T)
    keep = (~attn_mask.astype(bool)).astype(np.uint8)  # [B, q, ks]

    in_maps = []
    for c in range(8):
        b, half = c // 2, c % 2
        qs = half * SQ
        qt_full = np.ascontiguousarray(Q[b].T)           # [D, S]
        in_maps.append({
            "qt": np.ascontiguousarray(qt_full[:, qs:qs + SQ]),
            "kin": np.ascontiguousarray(K[b].T),
            "vin": np.ascontiguousarray(V[b].T),
            "qres": np.ascontiguousarray(Q[b, qs:qs + SQ, :]),
            "keept": np.ascontiguousarray(keep[b, qs:qs + SQ, :].T),
            "wqt": wqt, "wkt": wkt, "wvt": wvt, "wft": wft,
        })
    return in_maps


def kernel(Q, K, V, attn_mask, W_Q, W_K, W_V, W_fc):
    nc = _get_nc()
    in_maps = _make_in_maps(Q, K, V, attn_mask, W_Q, W_K, W_V, W_fc)
    res = run_bass_kernel_spmd(nc, in_maps, core_ids=list(range(8)))
    full = np.empty((B, S, D), dtype=np.float32)
    for c in range(8):
        b, half = c // 2, c % 2
        full[b, half * SQ:(half + 1) * SQ, :] = res.results[c]["out"]
    return full


def run_traced(Q, K, V, attn_mask, W_Q, W_K, W_V, W_fc, **kw):
    """Like kernel() but returns (full_output, BassKernelResults) with trace."""
    nc = _get_nc()
    in_maps = _make_in_maps(Q, K, V, attn_mask, W_Q, W_K, W_V, W_fc)
    res = run_bass_kernel_spmd(nc, in_maps, core_ids=list(range(8)), trace=True, **kw)
    full = np.empty((B, S, D), dtype=np.float32)
    for c in range(8):
        b, half = c // 2, c % 2
        full[b, half * SQ:(half + 1) * SQ, :] = res.results[c]["out"]
    return full, res
